# revision 50
# baseline (speedup 1.0000x reference)
"""Trainium2 Bass kernel for nn_Attention_53334903882008 (additive attention), v17.

Reference (per batch b):
  We  = img @ W^T + Wb;  Ue = (hid @ U^T + Ub) broadcast over T
  att = tanh(We + Ue);   e = att @ w + wb
  alpha = softmax_N(e);  phi = sum_n alpha * img      -> [B, T, D]

Sharding: data-parallel over B=8, one batch per NeuronCore; weights
replicated.

v17 over v16 (122.3 us): the v16 trace showed the DVE 98% busy — its
two 512-col ops per tile (descale+add-U, e-dot) became the wall while
the PE sat at 84%. Shift the balance:
  - The We matmul goes FULL fp8 DoubleRow (all 8 k-tiles; fp8 moving
    streams 2 cols/cycle, so each DR MM costs ~115 ns vs 216 for a
    bf16 k-tile pair): We = 4 DR streams.
  - The U_comb add moves from the DVE onto the PE: each tile's PSUM
    accumulation group OPENS with I128^T @ (16*U_comb) (one 512-col
    fp16 stream), and tanh reads PSUM directly with scale=1/16.
    The DVE keeps only the e-dot (+accumulator drain): ~0.78 us/tile.
  - The non-fp8 stack (xn, att, adiag, U_comb, w, base_log) moves
    bf16 -> fp16 (3 more mantissa bits, same byte cost) to buy back
    quantization margin: simulated end-to-end rel err 0.0192 vs the
    2e-2 gate (bf16 stack + full fp8 would be 0.0194).
Per-tile engine budget: PE ~1.14 us (ucomb + 4 DR + 2 phi + s), ACT
~1.09 us (tanh + exp), DVE ~0.78 us.

DMA: ONE ring (sync), strict need-order FIFO. Cross-queue engine
arbitration is coarse: with two HWDGE rings, whichever has backlog
monopolizes the 16-engine pool in multi-us bursts and the other
stream's needed-now bytes stall the PE (v14/v15 traces). A single
FIFO ordered by first-need paces itself (~417 GB/s measured). Unit =
4 btn-tiles: xt 0.5 MB, xn 1 MB; buffer-granular deps wake the PE per
4 tiles. Startup pieces (2+2+4 tiles) are separate SBUF tiles so
We(0) starts after ~0.9 MB.

Per 128-row btn-tile (64 tiles):
  - PSUM[btn, h] = I128^T @ ucomb16 + sum_g xt8^T @ wt8 (4 fp8-e4m3
    DoubleRow MMs; W pre-scaled by 16 host-side to clear the e4m3
    subnormal range) = 16*(We + U_comb)
  - tanh on ACT with scale=1/16 -> att fp16
  - e column [128,1] via one fused DVE scalar_tensor_tensor
    (out = att * w_rep, accum_out = sum_h)
  - adiag = exp(base_log + e) in ONE ACT op (bias = e per-partition;
    base_log is 0 on the block-diag band, -30000 off-band) -> the
    block-diagonal unnormalized-softmax matrix directly
  - phi[t,d] += adiag^T @ xn (2 fp16 matmuls) and s[t] += adiag^T @
    ones (N=1 matmul) accumulate in persistent PSUM over all 64 tiles
  - final: phi *= 1/s, DMA out.
U_comb = hid @ U^T + Wb + Ub (0.3% of the FLOPs) is computed host-side
in f32 and shipped in the blob as 16*U_comb fp16.
"""

from contextlib import ExitStack

import numpy as np
import ml_dtypes

import concourse.bacc as bacc
import concourse.tile as tile
from concourse import mybir
from concourse.bass_utils import run_bass_kernel_spmd

B = 8

BF = mybir.dt.bfloat16
F16 = mybir.dt.float16
F8 = mybir.dt.float8e4
U8 = mybir.dt.uint8
F32 = mybir.dt.float32
NPBF = ml_dtypes.bfloat16
NPF8 = ml_dtypes.float8_e4m3
DR = mybir.MatmulPerfMode.DoubleRow

T, N, D, H = 128, 64, 1024, 512
BTN = T * N            # 8192
NI = BTN // 128        # 64 btn-tiles of 128 rows
KT = D // 128          # 8 contraction tiles over d
G8 = 4                 # DoubleRow double-k groups (all kt in fp8)
WS = 16.0              # host-side W scale (keeps W out of e4m3 subnormals)
CPT = 8                # btn-tiles per chunk (startup bookkeeping)
PHI_LAG = 3            # tiles between chain(i) and phi(i) in PE order
N_WARM = 6             # garbage warmup MMs (HAM warm + DMA spin-up cover)

TC = G8 * 2 * 128      # 1024 fp8 bytes per tile in xt

# blob column offsets (fp16 [128, BLOB_C])
O_UCOMB = 0                     # [128, 512]  hid@U^T + Wb + Ub
O_WREP = O_UCOMB + H            # [128, 512]  w replicated over partitions
O_BASE = O_WREP + H             # [128, 254]  base_log band
O_ONEC = O_BASE + 254           # [128, 1]    ones column
BLOB_C = O_ONEC + 1


def build(nc):
    xt_d = nc.dram_tensor("xt", [128, NI * TC], U8, kind="ExternalInput").ap()
    xn_d = nc.dram_tensor("xn", [128, NI * D], F16, kind="ExternalInput").ap()
    wt8_d = nc.dram_tensor("wt8", [128, G8 * 2 * H], F8, kind="ExternalInput").ap()
    blob_d = nc.dram_tensor("blob", [128, BLOB_C], F16, kind="ExternalInput").ap()
    phi_d = nc.dram_tensor("phi", [T, D], F32, kind="ExternalOutput").ap()

    with tile.TileContext(nc) as tc, ExitStack() as ctx:
        consts = ctx.enter_context(tc.tile_pool(name="consts", bufs=1))
        xtp = ctx.enter_context(tc.tile_pool(name="xt", bufs=6))
        xnp = ctx.enter_context(tc.tile_pool(name="xn", bufs=5))
        attp = ctx.enter_context(tc.tile_pool(name="att", bufs=3))
        smal = ctx.enter_context(tc.tile_pool(name="smalls", bufs=6))
        pswe = ctx.enter_context(tc.tile_pool(name="pswe", bufs=5, space="PSUM"))
        psph = ctx.enter_context(tc.tile_pool(name="psphi", bufs=1, space="PSUM"))

        # ---- ONE ring (sync), strict need-order FIFO ----
        NU = NI // 4       # 16 units of 4 btn-tiles

        # Startup order solved against need-times at ~400 GB/s:
        # [xt0p0, wt8, xt0p1, blob, xt0p2, xn0a, xtU2, xn0b, xtU3,
        #  xnU2, xtU4, xnU3, xtU5]
        xt0_pieces = []   # (tile, first_ig, ntiles)
        t0a = consts.tile([128, 2, TC], U8)
        nc.sync.dma_start(out=t0a, in_=xt_d[:, 0 : 2 * TC])
        xt0_pieces.append((t0a, 0, 2))
        wt8_r = wt8_d.rearrange("p (g i h) -> p g i h", g=G8, i=2)
        # two tiles: the first DR MMs only wait on groups 0-1 (tile deps
        # are buffer-granular)
        wt8a = consts.tile([128, 2, 2, H], F8)
        nc.sync.dma_start(out=wt8a, in_=wt8_r[:, 0:2])
        t0b = consts.tile([128, 2, TC], U8)
        nc.sync.dma_start(out=t0b, in_=xt_d[:, 2 * TC : 4 * TC])
        xt0_pieces.append((t0b, 2, 2))
        wt8b = consts.tile([128, 2, 2, H], F8)
        nc.sync.dma_start(out=wt8b, in_=wt8_r[:, 2:4])
        wt8g = [wt8a[:, 0], wt8a[:, 1], wt8b[:, 0], wt8b[:, 1]]
        blob = consts.tile([128, BLOB_C], F16)
        nc.sync.dma_start(out=blob, in_=blob_d)
        t0c = consts.tile([128, 4, TC], U8)
        nc.sync.dma_start(out=t0c, in_=xt_d[:, 4 * TC : 8 * TC])
        xt0_pieces.append((t0c, 4, 4))

        def emit_xt(u):  # unit u covers btn-tiles 4u..4u+3
            xtc = xtp.tile([128, 4, TC], U8, tag="xt")
            nc.sync.dma_start(out=xtc, in_=xt_d[:, 4 * u * TC : 4 * (u + 1) * TC])
            return xtc

        def emit_xn(u):
            xnc = xnp.tile([128, 4, D], F16, tag="xn")
            nc.sync.dma_start(out=xnc, in_=xn_d[:, 4 * u * D : 4 * (u + 1) * D])
            return xnc

        xn0_pieces = []
        n0 = consts.tile([128, 4, D], F16)
        nc.sync.dma_start(out=n0, in_=xn_d[:, 0 : 4 * D])
        xn0_pieces.append(n0)
        xt_bufs, xn_bufs = {}, {}
        xt_bufs[2] = emit_xt(2)
        n1 = consts.tile([128, 4, D], F16)
        nc.sync.dma_start(out=n1, in_=xn_d[:, 4 * D : 8 * D])
        xn0_pieces.append(n1)
        xt_bufs[3] = emit_xt(3)
        xn_bufs[2] = emit_xn(2)
        xt_bufs[4] = emit_xt(4)
        xn_bufs[3] = emit_xn(3)
        xt_bufs[5] = emit_xt(5)

        def xt_view(ig):
            if ig < CPT:
                for t0, first, n in xt0_pieces:
                    if first <= ig < first + n:
                        return t0, ig - first
            return xt_bufs[ig // 4], ig % 4

        def xn_view(ig):
            if ig < CPT:
                return xn0_pieces[ig // 4], ig % 4
            return xn_bufs[ig // 4], ig % 4

        scratch = consts.tile([128, H], BF)  # HAM warmup fuel
        nc.gpsimd.memset(scratch, 0.0)

        ucomb = blob[:, O_UCOMB : O_UCOMB + H]
        w_rep = blob[:, O_WREP : O_WREP + H]
        base_log = blob[:, O_BASE : O_BASE + 254]
        onescol = blob[:, O_ONEC : O_ONEC + 1]

        # ---- persistent accumulators ----
        ps_phi0 = psph.tile([T, 512], F32, tag="phi0")
        ps_phi1 = psph.tile([T, 512], F32, tag="phi1")
        ps_phi = [ps_phi0, ps_phi1]
        ps_s = psph.tile([T, 1], F32, tag="s")

        def emit_we(ig):
            xtc, j = xt_view(ig)
            ps = pswe.tile([128, H], F32, tag="we")
            x8 = xtc[:, j, :].bitcast(F8).rearrange("p (g i c) -> p g i c", g=G8, i=2)
            for g in range(G8):
                m = nc.tensor.matmul(
                    ps, lhsT=x8[:, g], rhs=wt8g[g],
                    start=(g == 0), stop=(g == G8 - 1), perf_mode=DR,
                )
            return ps, m

        def emit_chain(ig, ps):
            # ps holds 16*We; descale and add U_comb in one DVE op, writing
            # to SBUF f32 so the PSUM bank frees here (not after tanh).
            ws32 = attp.tile([128, H], F32, tag="ws32")
            nc.vector.scalar_tensor_tensor(
                out=ws32, in0=ps, scalar=1.0 / WS, in1=ucomb,
                op0=mybir.AluOpType.mult, op1=mybir.AluOpType.add,
            )
            att = attp.tile([128, H], F16, tag="att")
            nc.scalar.activation(att, ws32, mybir.ActivationFunctionType.Tanh)
            scr = attp.tile([128, H], F16, tag="scr")
            ecol = smal.tile([128, 1], F32, tag="ecol")
            nc.vector.scalar_tensor_tensor(
                out=scr, in0=att, scalar=1.0, in1=w_rep,
                op0=mybir.AluOpType.mult, op1=mybir.AluOpType.mult,
                accum_out=ecol,
            )
            adiag = smal.tile([128, 128], F16, tag="adiag")
            nc.scalar.activation(
                adiag,
                base_log[:, 126 - 2 * ig : 254 - 2 * ig],
                mybir.ActivationFunctionType.Exp,
                bias=ecol,
            )
            return adiag

        def emit_phi(item):
            ig, adiag = item
            xnc, j = xn_view(ig)
            # s first: the finalize's reciprocal only needs ps_s, so the
            # last tile's s-stop lands before its phi pair
            nc.tensor.matmul(
                ps_s, lhsT=adiag, rhs=onescol, start=(ig == 0), stop=(ig == NI - 1)
            )
            for dh in range(2):
                nc.tensor.matmul(
                    ps_phi[dh],
                    lhsT=adiag,
                    rhs=xnc[:, j, dh * 512 : (dh + 1) * 512],
                    start=(ig == 0), stop=(ig == NI - 1),
                )

        # ---- main pipeline ----
        ps_warm = pswe.tile([128, H], F32, tag="we")
        for _ in range(N_WARM):
            nc.tensor.matmul(ps_warm, lhsT=scratch[:, 0:128], rhs=scratch,
                             start=True, stop=True)
        phi_pend = []  # (ig, adiag) awaiting phi emission
        for ig in range(NI):
            if ig % 4 == 0:
                u = ig // 4
                if u + 6 < NU:
                    xt_bufs[u + 6] = emit_xt(u + 6)
                if 4 <= u + 4 < NU:
                    xn_bufs[u + 4] = emit_xn(u + 4)
            ps, m_we = emit_we(ig)
            phi_pend.append((ig, emit_chain(ig, ps)))
            maxlag = 6 if ig < 16 else (PHI_LAG if ig < NI - 4 else 2)
            while len(phi_pend) > maxlag:
                emit_phi(phi_pend.pop(0))
        for item in phi_pend:
            emit_phi(item)

        # ---- finalize: phi = ps_phi * (1/s_t) ----
        recip = smal.tile([128, 1], F32, tag="recip")
        nc.vector.reciprocal(recip, ps_s)
        phi_sb = consts.tile([T, D], F32)
        # the two 1/s scales run on different engines so they overlap
        nc.vector.tensor_scalar_mul(phi_sb[:, 0:512], ps_phi[0], recip)
        nc.scalar.activation(
            phi_sb[:, 512:1024], ps_phi[1],
            mybir.ActivationFunctionType.Copy, scale=recip,
        )
        for dh in range(2):
            nc.sync.dma_start(
                out=phi_d[:, dh * 512 : (dh + 1) * 512],
                in_=phi_sb[:, dh * 512 : (dh + 1) * 512],
            )

    return nc


def prep_consts(W_weight, W_bias, U_weight, U_bias, w_weight):
    def pack_T(M):  # [H, D] -> [128, KT, H] f32, [p, kt, h] = M[h, kt*128+p]
        return M.T.astype(np.float32).reshape(KT, 128, H).transpose(1, 0, 2)

    wkt = pack_T(W_weight) * WS            # [128, KT, H], scaled
    wt8 = np.ascontiguousarray(wkt).reshape(128, G8 * 2 * H)
    wt8 = np.clip(wt8, -240, 240).astype(NPF8)

    blob = np.zeros((128, BLOB_C), np.float32)
    # ucomb filled per-core in prep_in_maps
    blob[:, O_WREP : O_WREP + H] = w_weight[0][None, :]
    blob[:, O_BASE : O_BASE + 254] = -30000.0
    for p in range(128):
        blob[p, O_BASE + 126 + p // 64] = 0.0
    blob[:, O_ONEC] = 1.0
    return {"wt8": wt8, "_blob_f32": blob}


_NC_CACHE = {}


def make_nc(num_devices=B):
    if num_devices not in _NC_CACHE:
        nc = bacc.Bacc(
            "TRN2", target_bir_lowering=False, debug=False, num_devices=num_devices
        )
        build(nc)
        nc.compile()
        _NC_CACHE[num_devices] = nc
    return _NC_CACHE[num_devices]


def prep_in_maps(img_features, hidden_state, consts):
    maps = []
    for b in range(B):
        xb = np.asarray(img_features[b], dtype=np.float32).reshape(BTN, D)
        xn = np.ascontiguousarray(
            xb.astype(np.float16).reshape(NI, 128, D).transpose(1, 0, 2)
        ).reshape(128, NI * D)
        # xt[p, tile, kt, c] = x[tile*128+c, kt*128+p] in fp8 bytes
        xkt = xb.reshape(NI, 128, KT, 128)
        x8 = np.clip(xkt.transpose(3, 0, 2, 1), -240, 240).astype(NPF8)
        xt = np.ascontiguousarray(x8.reshape(128, NI * TC).view(np.uint8))
        blob = consts["_blob_f32"].copy()
        # U_comb[c, h] = hid[c%64] @ U^T + Wb + Ub, f32 on host
        uc = consts["_ucomb_all"][b]
        blob[:, O_UCOMB : O_UCOMB + H] = np.concatenate([uc, uc], axis=0)
        maps.append(
            {
                "xt": xt, "xn": xn, "wt8": consts["wt8"],
                "blob": blob.astype(np.float16),
            }
        )
    return maps


def run(inputs, trace=False, tmpdir=None):
    """Run the SPMD kernel; returns (phi [B,T,D] fp32, BassKernelResults)."""
    inputs = {k: np.asarray(v) for k, v in inputs.items()}
    consts = prep_consts(
        inputs["W_weight"], inputs["W_bias"], inputs["U_weight"], inputs["U_bias"],
        inputs["w_weight"],
    )
    # [B, 64, H] = hid[n, b] @ U^T + (Wb + Ub)
    hid = np.asarray(inputs["hidden_state"], dtype=np.float32)
    consts["_ucomb_all"] = (
        np.einsum("nbd,hd->bnh", hid, inputs["U_weight"].astype(np.float32))
        + (inputs["W_bias"] + inputs["U_bias"]).astype(np.float32)
    )
    in_maps = prep_in_maps(inputs["img_features"], inputs["hidden_state"], consts)
    nc = make_nc(B)
    last_err = None
    for attempt in range(3):
        try:
            res = run_bass_kernel_spmd(
                nc, in_maps, core_ids=list(range(B)), trace=trace, tmpdir=tmpdir
            )
            break
        except Exception as e:  # transient NRT_EXEC_UNIT_UNRECOVERABLE etc.
            last_err = e
            if "UNRECOVERABLE" not in str(e) and "UNAVAILABLE" not in str(e):
                raise
    else:
        raise last_err
    phi = np.stack([res.results[b]["phi"] for b in range(B)]).astype(np.float32)
    return phi, res


def kernel(**inputs) -> np.ndarray:
    phi, _ = run(inputs, trace=False)
    return phi


# revision 52
# speedup vs baseline: 1.1661x; 1.1661x over previous
"""Trainium2 Bass kernel for nn_Attention_53334903882008 (additive attention), v17.

Reference (per batch b):
  We  = img @ W^T + Wb;  Ue = (hid @ U^T + Ub) broadcast over T
  att = tanh(We + Ue);   e = att @ w + wb
  alpha = softmax_N(e);  phi = sum_n alpha * img      -> [B, T, D]

Sharding: data-parallel over B=8, one batch per NeuronCore; weights
replicated.

v17 over v16 (122.3 us): the v16 trace showed the DVE 98% busy — its
two 512-col ops per tile (descale+add-U, e-dot) became the wall while
the PE sat at 84%. Shift the balance:
  - The We matmul goes FULL fp8 DoubleRow (all 8 k-tiles; fp8 moving
    streams 2 cols/cycle, so each DR MM costs ~115 ns vs 216 for a
    bf16 k-tile pair): We = 4 DR streams.
  - The U_comb add moves from the DVE onto the PE: each tile's PSUM
    accumulation group OPENS with I128^T @ (16*U_comb) (one 512-col
    fp16 stream), and tanh reads PSUM directly with scale=1/16.
    The DVE keeps only the e-dot (+accumulator drain): ~0.78 us/tile.
  - The non-fp8 stack (xn, att, adiag, U_comb, w, base_log) moves
    bf16 -> fp16 (3 more mantissa bits, same byte cost) to buy back
    quantization margin: simulated end-to-end rel err 0.0192 vs the
    2e-2 gate (bf16 stack + full fp8 would be 0.0194).
Per-tile engine budget: PE ~1.14 us (ucomb + 4 DR + 2 phi + s), ACT
~1.09 us (tanh + exp), DVE ~0.78 us.

DMA: ONE ring (sync), strict need-order FIFO. Cross-queue engine
arbitration is coarse: with two HWDGE rings, whichever has backlog
monopolizes the 16-engine pool in multi-us bursts and the other
stream's needed-now bytes stall the PE (v14/v15 traces). A single
FIFO ordered by first-need paces itself (~417 GB/s measured). Unit =
4 btn-tiles: xt 0.5 MB, xn 1 MB; buffer-granular deps wake the PE per
4 tiles. Startup pieces (2+2+4 tiles) are separate SBUF tiles so
We(0) starts after ~0.9 MB.

Per 128-row btn-tile (64 tiles):
  - PSUM[btn, h] = I128^T @ ucomb16 + sum_g xt8^T @ wt8 (4 fp8-e4m3
    DoubleRow MMs; W pre-scaled by 16 host-side to clear the e4m3
    subnormal range) = 16*(We + U_comb)
  - tanh on ACT with scale=1/16 -> att fp16
  - e column [128,1] via one fused DVE scalar_tensor_tensor
    (out = att * w_rep, accum_out = sum_h)
  - adiag = exp(base_log + e) in ONE ACT op (bias = e per-partition;
    base_log is 0 on the block-diag band, -30000 off-band) -> the
    block-diagonal unnormalized-softmax matrix directly
  - phi[t,d] += adiag^T @ xn (2 fp16 matmuls) and s[t] += adiag^T @
    ones (N=1 matmul) accumulate in persistent PSUM over all 64 tiles
  - final: phi *= 1/s, DMA out.
U_comb = hid @ U^T + Wb + Ub (0.3% of the FLOPs) is computed host-side
in f32 and shipped in the blob as 16*U_comb fp16.
"""

from contextlib import ExitStack

import numpy as np
import ml_dtypes

import concourse.bacc as bacc
import concourse.tile as tile
from concourse import mybir
from concourse.bass_utils import run_bass_kernel_spmd

B = 8

BF = mybir.dt.bfloat16
F16 = mybir.dt.float16
F8 = mybir.dt.float8e4
U8 = mybir.dt.uint8
F32 = mybir.dt.float32
NPBF = ml_dtypes.bfloat16
NPF8 = ml_dtypes.float8_e4m3
DR = mybir.MatmulPerfMode.DoubleRow

T, N, D, H = 128, 64, 1024, 512
BTN = T * N            # 8192
NI = BTN // 128        # 64 btn-tiles of 128 rows
KT = D // 128          # 8 contraction tiles over d
G8 = 4                 # DoubleRow double-k groups (all kt in fp8)
WS = 16.0              # host-side W scale (keeps W out of e4m3 subnormals)
CPT = 8                # btn-tiles per chunk (startup bookkeeping)
PHI_LAG = 3            # tiles between chain(i) and phi(i) in PE order
N_WARM = 12            # garbage warmup MMs (HAM warm + DMA spin-up cover)

TC = G8 * 2 * 128      # 1024 fp8 bytes per tile in xt

# blob column offsets (fp16 [128, BLOB_C])
O_UCOMB = 0                     # [128, 512]  hid@U^T + Wb + Ub
O_WREP = O_UCOMB + H            # [128, 512]  w replicated over partitions
O_BASE = O_WREP + H             # [128, 254]  base_log band
O_ONEC = O_BASE + 254           # [128, 1]    ones column
BLOB_C = O_ONEC + 1


def build(nc):
    xt_d = nc.dram_tensor("xt", [128, NI * TC], U8, kind="ExternalInput").ap()
    xn_d = nc.dram_tensor("xn", [128, NI * D], F16, kind="ExternalInput").ap()
    wt8_d = nc.dram_tensor("wt8", [128, G8 * 2 * H], F8, kind="ExternalInput").ap()
    blob_d = nc.dram_tensor("blob", [128, BLOB_C], F16, kind="ExternalInput").ap()
    phi_d = nc.dram_tensor("phi", [T, D], F32, kind="ExternalOutput").ap()

    with tile.TileContext(nc) as tc, ExitStack() as ctx:
        consts = ctx.enter_context(tc.tile_pool(name="consts", bufs=1))
        xtp = ctx.enter_context(tc.tile_pool(name="xt", bufs=6))
        xnp = ctx.enter_context(tc.tile_pool(name="xn", bufs=5))
        attp = ctx.enter_context(tc.tile_pool(name="att", bufs=3))
        smal = ctx.enter_context(tc.tile_pool(name="smalls", bufs=6))
        pswe = ctx.enter_context(tc.tile_pool(name="pswe", bufs=5, space="PSUM"))
        psph = ctx.enter_context(tc.tile_pool(name="psphi", bufs=1, space="PSUM"))

        # ---- ONE ring (sync), strict need-order FIFO ----
        NU = NI // 4       # 16 units of 4 btn-tiles

        # Startup order solved against need-times at ~400 GB/s:
        # [xt0p0, wt8, xt0p1, blob, xt0p2, xn0a, xtU2, xn0b, xtU3,
        #  xnU2, xtU4, xnU3, xtU5]
        xt0_pieces = []   # (tile, first_ig, ntiles)
        t0a = consts.tile([128, 2, TC], U8)
        nc.sync.dma_start(out=t0a, in_=xt_d[:, 0 : 2 * TC])
        xt0_pieces.append((t0a, 0, 2))
        wt8 = consts.tile([128, G8, 2, H], F8)
        nc.sync.dma_start(out=wt8, in_=wt8_d.rearrange("p (g i h) -> p g i h", g=G8, i=2))
        t0b = consts.tile([128, 2, TC], U8)
        nc.sync.dma_start(out=t0b, in_=xt_d[:, 2 * TC : 4 * TC])
        xt0_pieces.append((t0b, 2, 2))
        wt8g = [wt8[:, g] for g in range(G8)]
        blob = consts.tile([128, BLOB_C], F16)
        nc.sync.dma_start(out=blob, in_=blob_d)
        t0c = consts.tile([128, 4, TC], U8)
        nc.sync.dma_start(out=t0c, in_=xt_d[:, 4 * TC : 8 * TC])
        xt0_pieces.append((t0c, 4, 4))

        def emit_xt(u):  # unit u covers btn-tiles 4u..4u+3
            xtc = xtp.tile([128, 4, TC], U8, tag="xt")
            nc.sync.dma_start(out=xtc, in_=xt_d[:, 4 * u * TC : 4 * (u + 1) * TC])
            return xtc

        def emit_xn(u):
            xnc = xnp.tile([128, 4, D], F16, tag="xn")
            nc.sync.dma_start(out=xnc, in_=xn_d[:, 4 * u * D : 4 * (u + 1) * D])
            return xnc

        xn0_pieces = []
        n0 = consts.tile([128, 4, D], F16)
        nc.sync.dma_start(out=n0, in_=xn_d[:, 0 : 4 * D])
        xn0_pieces.append(n0)
        xt_bufs, xn_bufs = {}, {}
        xt_bufs[2] = emit_xt(2)
        n1 = consts.tile([128, 4, D], F16)
        nc.sync.dma_start(out=n1, in_=xn_d[:, 4 * D : 8 * D])
        xn0_pieces.append(n1)
        xt_bufs[3] = emit_xt(3)
        xn_bufs[2] = emit_xn(2)
        xt_bufs[4] = emit_xt(4)
        xn_bufs[3] = emit_xn(3)
        xt_bufs[5] = emit_xt(5)

        def xt_view(ig):
            if ig < CPT:
                for t0, first, n in xt0_pieces:
                    if first <= ig < first + n:
                        return t0, ig - first
            return xt_bufs[ig // 4], ig % 4

        def xn_view(ig):
            if ig < CPT:
                return xn0_pieces[ig // 4], ig % 4
            return xn_bufs[ig // 4], ig % 4

        scratch = consts.tile([128, H], BF)  # HAM warmup fuel
        nc.gpsimd.memset(scratch, 0.0)

        ucomb = blob[:, O_UCOMB : O_UCOMB + H]
        w_rep = blob[:, O_WREP : O_WREP + H]
        base_log = blob[:, O_BASE : O_BASE + 254]
        onescol = blob[:, O_ONEC : O_ONEC + 1]

        # ---- persistent accumulators ----
        ps_phi0 = psph.tile([T, 512], F32, tag="phi0")
        ps_phi1 = psph.tile([T, 512], F32, tag="phi1")
        ps_phi = [ps_phi0, ps_phi1]
        ps_s = psph.tile([T, 1], F32, tag="s")

        def emit_we(ig):
            xtc, j = xt_view(ig)
            ps = pswe.tile([128, H], F32, tag="we")
            x8 = xtc[:, j, :].bitcast(F8).rearrange("p (g i c) -> p g i c", g=G8, i=2)
            for g in range(G8):
                m = nc.tensor.matmul(
                    ps, lhsT=x8[:, g], rhs=wt8g[g],
                    start=(g == 0), stop=(g == G8 - 1), perf_mode=DR,
                )
            return ps, m

        def emit_chain(ig, ps):
            # ps holds 16*We; descale and add U_comb in one DVE op, writing
            # to SBUF f32 so the PSUM bank frees here (not after tanh).
            ws32 = attp.tile([128, H], F32, tag="ws32")
            nc.vector.scalar_tensor_tensor(
                out=ws32, in0=ps, scalar=1.0 / WS, in1=ucomb,
                op0=mybir.AluOpType.mult, op1=mybir.AluOpType.add,
            )
            att = attp.tile([128, H], F16, tag="att")
            nc.scalar.activation(att, ws32, mybir.ActivationFunctionType.Tanh)
            scr = attp.tile([128, H], F16, tag="scr")
            ecol = smal.tile([128, 1], F32, tag="ecol")
            nc.vector.scalar_tensor_tensor(
                out=scr, in0=att, scalar=1.0, in1=w_rep,
                op0=mybir.AluOpType.mult, op1=mybir.AluOpType.mult,
                accum_out=ecol,
            )
            adiag = smal.tile([128, 128], F16, tag="adiag")
            nc.scalar.activation(
                adiag,
                base_log[:, 126 - 2 * ig : 254 - 2 * ig],
                mybir.ActivationFunctionType.Exp,
                bias=ecol,
            )
            return adiag

        def emit_phi(item):
            ig, adiag = item
            xnc, j = xn_view(ig)
            # s first: the finalize's reciprocal only needs ps_s, so the
            # last tile's s-stop lands before its phi pair
            nc.tensor.matmul(
                ps_s, lhsT=adiag, rhs=onescol, start=(ig == 0), stop=(ig == NI - 1)
            )
            for dh in range(2):
                nc.tensor.matmul(
                    ps_phi[dh],
                    lhsT=adiag,
                    rhs=xnc[:, j, dh * 512 : (dh + 1) * 512],
                    start=(ig == 0), stop=(ig == NI - 1),
                )

        # ---- main pipeline ----
        ps_warm = pswe.tile([128, H], F32, tag="we")
        for _ in range(N_WARM):
            nc.tensor.matmul(ps_warm, lhsT=scratch[:, 0:128], rhs=scratch,
                             start=True, stop=True)
        phi_pend = []  # (ig, adiag) awaiting phi emission
        for ig in range(NI):
            if ig % 4 == 0:
                u = ig // 4
                if u + 6 < NU:
                    xt_bufs[u + 6] = emit_xt(u + 6)
                if 4 <= u + 4 < NU:
                    xn_bufs[u + 4] = emit_xn(u + 4)
            ps, m_we = emit_we(ig)
            phi_pend.append((ig, emit_chain(ig, ps)))
            maxlag = 6 if ig < 16 else (PHI_LAG if ig < NI - 4 else 2)
            while len(phi_pend) > maxlag:
                emit_phi(phi_pend.pop(0))
        for item in phi_pend:
            emit_phi(item)

        # ---- finalize: phi = ps_phi * (1/s_t) ----
        recip = smal.tile([128, 1], F32, tag="recip")
        nc.vector.reciprocal(recip, ps_s)
        phi_sb = consts.tile([T, D], F32)
        # the two 1/s scales run on different engines so they overlap
        nc.vector.tensor_scalar_mul(phi_sb[:, 0:512], ps_phi[0], recip)
        nc.scalar.activation(
            phi_sb[:, 512:1024], ps_phi[1],
            mybir.ActivationFunctionType.Copy, scale=recip,
        )
        for dh in range(2):
            nc.sync.dma_start(
                out=phi_d[:, dh * 512 : (dh + 1) * 512],
                in_=phi_sb[:, dh * 512 : (dh + 1) * 512],
            )

    return nc


def prep_consts(W_weight, W_bias, U_weight, U_bias, w_weight):
    def pack_T(M):  # [H, D] -> [128, KT, H] f32, [p, kt, h] = M[h, kt*128+p]
        return M.T.astype(np.float32).reshape(KT, 128, H).transpose(1, 0, 2)

    wkt = pack_T(W_weight) * WS            # [128, KT, H], scaled
    wt8 = np.ascontiguousarray(wkt).reshape(128, G8 * 2 * H)
    wt8 = np.clip(wt8, -240, 240).astype(NPF8)

    blob = np.zeros((128, BLOB_C), np.float32)
    # ucomb filled per-core in prep_in_maps
    blob[:, O_WREP : O_WREP + H] = w_weight[0][None, :]
    blob[:, O_BASE : O_BASE + 254] = -30000.0
    for p in range(128):
        blob[p, O_BASE + 126 + p // 64] = 0.0
    blob[:, O_ONEC] = 1.0
    return {"wt8": wt8, "_blob_f32": blob}


_NC_CACHE = {}


def make_nc(num_devices=B):
    if num_devices not in _NC_CACHE:
        nc = bacc.Bacc(
            "TRN2", target_bir_lowering=False, debug=False, num_devices=num_devices
        )
        build(nc)
        nc.compile()
        _NC_CACHE[num_devices] = nc
    return _NC_CACHE[num_devices]


def prep_in_maps(img_features, hidden_state, consts):
    maps = []
    for b in range(B):
        xb = np.asarray(img_features[b], dtype=np.float32).reshape(BTN, D)
        xn = np.ascontiguousarray(
            xb.astype(np.float16).reshape(NI, 128, D).transpose(1, 0, 2)
        ).reshape(128, NI * D)
        # xt[p, tile, kt, c] = x[tile*128+c, kt*128+p] in fp8 bytes
        xkt = xb.reshape(NI, 128, KT, 128)
        x8 = np.clip(xkt.transpose(3, 0, 2, 1), -240, 240).astype(NPF8)
        xt = np.ascontiguousarray(x8.reshape(128, NI * TC).view(np.uint8))
        blob = consts["_blob_f32"].copy()
        # U_comb[c, h] = hid[c%64] @ U^T + Wb + Ub, f32 on host
        uc = consts["_ucomb_all"][b]
        blob[:, O_UCOMB : O_UCOMB + H] = np.concatenate([uc, uc], axis=0)
        maps.append(
            {
                "xt": xt, "xn": xn, "wt8": consts["wt8"],
                "blob": blob.astype(np.float16),
            }
        )
    return maps


def run(inputs, trace=False, tmpdir=None):
    """Run the SPMD kernel; returns (phi [B,T,D] fp32, BassKernelResults)."""
    inputs = {k: np.asarray(v) for k, v in inputs.items()}
    consts = prep_consts(
        inputs["W_weight"], inputs["W_bias"], inputs["U_weight"], inputs["U_bias"],
        inputs["w_weight"],
    )
    # [B, 64, H] = hid[n, b] @ U^T + (Wb + Ub)
    hid = np.asarray(inputs["hidden_state"], dtype=np.float32)
    consts["_ucomb_all"] = (
        np.einsum("nbd,hd->bnh", hid, inputs["U_weight"].astype(np.float32))
        + (inputs["W_bias"] + inputs["U_bias"]).astype(np.float32)
    )
    in_maps = prep_in_maps(inputs["img_features"], inputs["hidden_state"], consts)
    nc = make_nc(B)
    last_err = None
    for attempt in range(3):
        try:
            res = run_bass_kernel_spmd(
                nc, in_maps, core_ids=list(range(B)), trace=trace, tmpdir=tmpdir
            )
            break
        except Exception as e:  # transient NRT_EXEC_UNIT_UNRECOVERABLE etc.
            last_err = e
            if "UNRECOVERABLE" not in str(e) and "UNAVAILABLE" not in str(e):
                raise
    else:
        raise last_err
    phi = np.stack([res.results[b]["phi"] for b in range(B)]).astype(np.float32)
    return phi, res


def kernel(**inputs) -> np.ndarray:
    phi, _ = run(inputs, trace=False)
    return phi


# revision 53
# speedup vs baseline: 1.1743x; 1.0070x over previous
"""Trainium2 Bass kernel for nn_Attention_53334903882008 (additive attention), v20.

Reference (per batch b):
  We  = img @ W^T + Wb;  Ue = (hid @ U^T + Ub) broadcast over T
  att = tanh(We + Ue);   e = att @ w + wb
  alpha = softmax_N(e);  phi = sum_n alpha * img      -> [B, T, D]

Sharding: data-parallel over B=8, one batch per NeuronCore; weights
replicated. Measured 106.4-107.5 us (baseline v9 bf16: 169.9 us),
rel err 0.019245 (gate 2e-2) — deterministic across runs.

How the 1.6x was won (v10..v20):
  - The We matmul (8192x1024x512 per core, 64% of PE work) runs FULLY
    in fp8-e4m3 with perf_mode=DoubleRow: each [128,2,*] MM contracts
    256 rows in one ~512-cycle stream, so We costs 4 streams instead
    of 8. W is pre-scaled by 16 host-side so its ~N(0,1/1024) entries
    clear the e4m3 subnormal range; the 1/16 descale is folded into
    the DVE op that adds U_comb.
  - The non-fp8 stack (xn, att, adiag, U_comb, w, base_log) is fp16
    rather than bf16 (3 more mantissa bits, same bytes): buys the
    quantization margin that makes full-fp8 We fit the 2e-2 gate
    (0.0192 vs 0.0194 simulated; both match HW to ~2e-5).
  - U_comb = hid @ U^T + Wb + Ub (0.3% of FLOPs) is computed on the
    host and shipped in the blob, killing 9 setup matmuls and the
    startup stall waiting for it.
  - Engine balance per tile: PE ~1.3 us (4 DR + 2 phi + s), DVE
    ~1.4 us queue-overlapped (descale+add-U stt writing SBUF f32 so
    the PSUM bank frees early, e-dot stt with accum), ACT ~1.1 us
    (tanh + exp). PSUM: 5 We banks + phi0/phi1/s.
  - HAM warmup: 12 garbage MMs keep the PE busy through the DMA
    spin-up so the clock gate reaches 8/8 by ~13 us and never drops.

DMA: ONE ring (sync), strict need-order FIFO. Cross-queue engine
arbitration is coarse: with two HWDGE rings, whichever has backlog
monopolizes the 16-engine pool in multi-us bursts and the other
stream's needed-now bytes stall the PE (v14/v15 traces). A single
FIFO ordered by first-need paces itself (~417 GB/s measured). Unit =
4 btn-tiles: xt 0.5 MB, xn 1 MB; buffer-granular deps wake the PE per
4 tiles. Startup pieces (2+2+4 tiles) are separate SBUF tiles so
We(0) starts after ~0.9 MB.

Per 128-row btn-tile (64 tiles):
  - PSUM[btn, h] = I128^T @ ucomb16 + sum_g xt8^T @ wt8 (4 fp8-e4m3
    DoubleRow MMs; W pre-scaled by 16 host-side to clear the e4m3
    subnormal range) = 16*(We + U_comb)
  - tanh on ACT with scale=1/16 -> att fp16
  - e column [128,1] via one fused DVE scalar_tensor_tensor
    (out = att * w_rep, accum_out = sum_h)
  - adiag = exp(base_log + e) in ONE ACT op (bias = e per-partition;
    base_log is 0 on the block-diag band, -30000 off-band) -> the
    block-diagonal unnormalized-softmax matrix directly
  - phi[t,d] += adiag^T @ xn (2 fp16 matmuls) and s[t] += adiag^T @
    ones (N=1 matmul) accumulate in persistent PSUM over all 64 tiles
  - final: phi *= 1/s, DMA out.
U_comb = hid @ U^T + Wb + Ub (0.3% of the FLOPs) is computed host-side
in f32 and shipped in the blob as 16*U_comb fp16.
"""

from contextlib import ExitStack

import numpy as np
import ml_dtypes

import concourse.bacc as bacc
import concourse.tile as tile
from concourse import mybir
from concourse.bass_utils import run_bass_kernel_spmd

B = 8

BF = mybir.dt.bfloat16
F16 = mybir.dt.float16
F8 = mybir.dt.float8e4
U8 = mybir.dt.uint8
F32 = mybir.dt.float32
NPBF = ml_dtypes.bfloat16
NPF8 = ml_dtypes.float8_e4m3
DR = mybir.MatmulPerfMode.DoubleRow

T, N, D, H = 128, 64, 1024, 512
BTN = T * N            # 8192
NI = BTN // 128        # 64 btn-tiles of 128 rows
KT = D // 128          # 8 contraction tiles over d
G8 = 4                 # DoubleRow double-k groups (all kt in fp8)
WS = 16.0              # host-side W scale (keeps W out of e4m3 subnormals)
CPT = 8                # btn-tiles per chunk (startup bookkeeping)
PHI_LAG = 3            # tiles between chain(i) and phi(i) in PE order
N_WARM = 12            # garbage warmup MMs (HAM warm + DMA spin-up cover)

TC = G8 * 2 * 128      # 1024 fp8 bytes per tile in xt

# blob column offsets (fp16 [128, BLOB_C])
O_UCOMB = 0                     # [128, 512]  hid@U^T + Wb + Ub
O_WREP = O_UCOMB + H            # [128, 512]  w replicated over partitions
O_BASE = O_WREP + H             # [128, 254]  base_log band
O_ONEC = O_BASE + 254           # [128, 1]    ones column
BLOB_C = O_ONEC + 1


def build(nc):
    xt_d = nc.dram_tensor("xt", [128, NI * TC], U8, kind="ExternalInput").ap()
    xn_d = nc.dram_tensor("xn", [128, NI * D], F16, kind="ExternalInput").ap()
    wt8_d = nc.dram_tensor("wt8", [128, G8 * 2 * H], F8, kind="ExternalInput").ap()
    blob_d = nc.dram_tensor("blob", [128, BLOB_C], F16, kind="ExternalInput").ap()
    phi_d = nc.dram_tensor("phi", [T, D], F32, kind="ExternalOutput").ap()

    with tile.TileContext(nc) as tc, ExitStack() as ctx:
        consts = ctx.enter_context(tc.tile_pool(name="consts", bufs=1))
        xtp = ctx.enter_context(tc.tile_pool(name="xt", bufs=6))
        xnp = ctx.enter_context(tc.tile_pool(name="xn", bufs=5))
        attp = ctx.enter_context(tc.tile_pool(name="att", bufs=3))
        smal = ctx.enter_context(tc.tile_pool(name="smalls", bufs=6))
        pswe = ctx.enter_context(tc.tile_pool(name="pswe", bufs=5, space="PSUM"))
        psph = ctx.enter_context(tc.tile_pool(name="psphi", bufs=1, space="PSUM"))

        # ---- ONE ring (sync), strict need-order FIFO ----
        NU = NI // 4       # 16 units of 4 btn-tiles

        # Startup order solved against need-times at ~400 GB/s:
        # [xt0p0, wt8, xt0p1, blob, xt0p2, xn0a, xtU2, xn0b, xtU3,
        #  xnU2, xtU4, xnU3, xtU5]
        xt0_pieces = []   # (tile, first_ig, ntiles)
        t0a = consts.tile([128, 2, TC], U8)
        nc.sync.dma_start(out=t0a, in_=xt_d[:, 0 : 2 * TC])
        xt0_pieces.append((t0a, 0, 2))
        wt8 = consts.tile([128, G8, 2, H], F8)
        nc.sync.dma_start(out=wt8, in_=wt8_d.rearrange("p (g i h) -> p g i h", g=G8, i=2))
        t0b = consts.tile([128, 2, TC], U8)
        nc.sync.dma_start(out=t0b, in_=xt_d[:, 2 * TC : 4 * TC])
        xt0_pieces.append((t0b, 2, 2))
        wt8g = [wt8[:, g] for g in range(G8)]
        blob = consts.tile([128, BLOB_C], F16)
        nc.sync.dma_start(out=blob, in_=blob_d)
        t0c = consts.tile([128, 4, TC], U8)
        nc.sync.dma_start(out=t0c, in_=xt_d[:, 4 * TC : 8 * TC])
        xt0_pieces.append((t0c, 4, 4))

        def emit_xt(u):  # unit u covers btn-tiles 4u..4u+3
            xtc = xtp.tile([128, 4, TC], U8, tag="xt")
            nc.sync.dma_start(out=xtc, in_=xt_d[:, 4 * u * TC : 4 * (u + 1) * TC])
            return xtc

        def emit_xn(u):
            xnc = xnp.tile([128, 4, D], F16, tag="xn")
            nc.sync.dma_start(out=xnc, in_=xn_d[:, 4 * u * D : 4 * (u + 1) * D])
            return xnc

        xn0_pieces = []
        n0 = consts.tile([128, 4, D], F16)
        nc.sync.dma_start(out=n0, in_=xn_d[:, 0 : 4 * D])
        xn0_pieces.append(n0)
        xt_bufs, xn_bufs = {}, {}
        xt_bufs[2] = emit_xt(2)
        n1 = consts.tile([128, 4, D], F16)
        nc.sync.dma_start(out=n1, in_=xn_d[:, 4 * D : 8 * D])
        xn0_pieces.append(n1)
        xt_bufs[3] = emit_xt(3)
        xn_bufs[2] = emit_xn(2)
        xt_bufs[4] = emit_xt(4)
        xn_bufs[3] = emit_xn(3)
        xt_bufs[5] = emit_xt(5)

        def xt_view(ig):
            if ig < CPT:
                for t0, first, n in xt0_pieces:
                    if first <= ig < first + n:
                        return t0, ig - first
            return xt_bufs[ig // 4], ig % 4

        def xn_view(ig):
            if ig < CPT:
                return xn0_pieces[ig // 4], ig % 4
            return xn_bufs[ig // 4], ig % 4

        scratch = consts.tile([128, H], BF)  # HAM warmup fuel
        nc.gpsimd.memset(scratch, 0.0)

        ucomb = blob[:, O_UCOMB : O_UCOMB + H]
        w_rep = blob[:, O_WREP : O_WREP + H]
        base_log = blob[:, O_BASE : O_BASE + 254]
        onescol = blob[:, O_ONEC : O_ONEC + 1]

        # ---- persistent accumulators ----
        ps_phi0 = psph.tile([T, 512], F32, tag="phi0")
        ps_phi1 = psph.tile([T, 512], F32, tag="phi1")
        ps_phi = [ps_phi0, ps_phi1]
        ps_s = psph.tile([T, 1], F32, tag="s")

        def emit_we(ig):
            xtc, j = xt_view(ig)
            ps = pswe.tile([128, H], F32, tag="we")
            x8 = xtc[:, j, :].bitcast(F8).rearrange("p (g i c) -> p g i c", g=G8, i=2)
            for g in range(G8):
                m = nc.tensor.matmul(
                    ps, lhsT=x8[:, g], rhs=wt8g[g],
                    start=(g == 0), stop=(g == G8 - 1), perf_mode=DR,
                )
            return ps, m

        def emit_chain(ig, ps):
            # ps holds 16*We; descale and add U_comb in one DVE op, writing
            # to SBUF f32 so the PSUM bank frees here (not after tanh).
            ws32 = attp.tile([128, H], F32, tag="ws32")
            nc.vector.scalar_tensor_tensor(
                out=ws32, in0=ps, scalar=1.0 / WS, in1=ucomb,
                op0=mybir.AluOpType.mult, op1=mybir.AluOpType.add,
            )
            att = attp.tile([128, H], F16, tag="att")
            nc.scalar.activation(att, ws32, mybir.ActivationFunctionType.Tanh)
            scr = attp.tile([128, H], F16, tag="scr")
            ecol = smal.tile([128, 1], F32, tag="ecol")
            nc.vector.scalar_tensor_tensor(
                out=scr, in0=att, scalar=1.0, in1=w_rep,
                op0=mybir.AluOpType.mult, op1=mybir.AluOpType.mult,
                accum_out=ecol,
            )
            adiag = smal.tile([128, 128], F16, tag="adiag")
            nc.scalar.activation(
                adiag,
                base_log[:, 126 - 2 * ig : 254 - 2 * ig],
                mybir.ActivationFunctionType.Exp,
                bias=ecol,
            )
            return adiag

        def emit_phi(item):
            ig, adiag = item
            xnc, j = xn_view(ig)
            # s first: the finalize's reciprocal only needs ps_s, so the
            # last tile's s-stop lands before its phi pair
            nc.tensor.matmul(
                ps_s, lhsT=adiag, rhs=onescol, start=(ig == 0), stop=(ig == NI - 1)
            )
            for dh in range(2):
                nc.tensor.matmul(
                    ps_phi[dh],
                    lhsT=adiag,
                    rhs=xnc[:, j, dh * 512 : (dh + 1) * 512],
                    start=(ig == 0), stop=(ig == NI - 1),
                )

        # ---- main pipeline ----
        ps_warm = pswe.tile([128, H], F32, tag="we")
        for _ in range(N_WARM):
            nc.tensor.matmul(ps_warm, lhsT=scratch[:, 0:128], rhs=scratch,
                             start=True, stop=True)
        phi_pend = []  # (ig, adiag) awaiting phi emission
        for ig in range(NI):
            if ig % 4 == 0:
                u = ig // 4
                if u + 6 < NU:
                    xt_bufs[u + 6] = emit_xt(u + 6)
                if 4 <= u + 4 < NU:
                    xn_bufs[u + 4] = emit_xn(u + 4)
            ps, m_we = emit_we(ig)
            phi_pend.append((ig, emit_chain(ig, ps)))
            maxlag = 6 if ig < 16 else (PHI_LAG if ig < NI - 4 else 2)
            while len(phi_pend) > maxlag:
                emit_phi(phi_pend.pop(0))
        for item in phi_pend:
            emit_phi(item)

        # ---- finalize: phi = ps_phi * (1/s_t) ----
        recip = smal.tile([128, 1], F32, tag="recip")
        nc.vector.reciprocal(recip, ps_s)
        phi_sb = consts.tile([T, D], F32)
        # the two 1/s scales run on different engines so they overlap
        nc.vector.tensor_scalar_mul(phi_sb[:, 0:512], ps_phi[0], recip)
        nc.scalar.activation(
            phi_sb[:, 512:1024], ps_phi[1],
            mybir.ActivationFunctionType.Copy, scale=recip,
        )
        for dh in range(2):
            nc.sync.dma_start(
                out=phi_d[:, dh * 512 : (dh + 1) * 512],
                in_=phi_sb[:, dh * 512 : (dh + 1) * 512],
            )

    return nc


def prep_consts(W_weight, W_bias, U_weight, U_bias, w_weight):
    def pack_T(M):  # [H, D] -> [128, KT, H] f32, [p, kt, h] = M[h, kt*128+p]
        return M.T.astype(np.float32).reshape(KT, 128, H).transpose(1, 0, 2)

    wkt = pack_T(W_weight) * WS            # [128, KT, H], scaled
    wt8 = np.ascontiguousarray(wkt).reshape(128, G8 * 2 * H)
    wt8 = np.clip(wt8, -240, 240).astype(NPF8)

    blob = np.zeros((128, BLOB_C), np.float32)
    # ucomb filled per-core in prep_in_maps
    blob[:, O_WREP : O_WREP + H] = w_weight[0][None, :]
    blob[:, O_BASE : O_BASE + 254] = -30000.0
    for p in range(128):
        blob[p, O_BASE + 126 + p // 64] = 0.0
    blob[:, O_ONEC] = 1.0
    return {"wt8": wt8, "_blob_f32": blob}


_NC_CACHE = {}


def make_nc(num_devices=B):
    if num_devices not in _NC_CACHE:
        nc = bacc.Bacc(
            "TRN2", target_bir_lowering=False, debug=False, num_devices=num_devices
        )
        build(nc)
        nc.compile()
        _NC_CACHE[num_devices] = nc
    return _NC_CACHE[num_devices]


def prep_in_maps(img_features, hidden_state, consts):
    maps = []
    for b in range(B):
        xb = np.asarray(img_features[b], dtype=np.float32).reshape(BTN, D)
        xn = np.ascontiguousarray(
            xb.astype(np.float16).reshape(NI, 128, D).transpose(1, 0, 2)
        ).reshape(128, NI * D)
        # xt[p, tile, kt, c] = x[tile*128+c, kt*128+p] in fp8 bytes
        xkt = xb.reshape(NI, 128, KT, 128)
        x8 = np.clip(xkt.transpose(3, 0, 2, 1), -240, 240).astype(NPF8)
        xt = np.ascontiguousarray(x8.reshape(128, NI * TC).view(np.uint8))
        blob = consts["_blob_f32"].copy()
        # U_comb[c, h] = hid[c%64] @ U^T + Wb + Ub, f32 on host
        uc = consts["_ucomb_all"][b]
        blob[:, O_UCOMB : O_UCOMB + H] = np.concatenate([uc, uc], axis=0)
        maps.append(
            {
                "xt": xt, "xn": xn, "wt8": consts["wt8"],
                "blob": blob.astype(np.float16),
            }
        )
    return maps


def run(inputs, trace=False, tmpdir=None):
    """Run the SPMD kernel; returns (phi [B,T,D] fp32, BassKernelResults)."""
    inputs = {k: np.asarray(v) for k, v in inputs.items()}
    consts = prep_consts(
        inputs["W_weight"], inputs["W_bias"], inputs["U_weight"], inputs["U_bias"],
        inputs["w_weight"],
    )
    # [B, 64, H] = hid[n, b] @ U^T + (Wb + Ub)
    hid = np.asarray(inputs["hidden_state"], dtype=np.float32)
    consts["_ucomb_all"] = (
        np.einsum("nbd,hd->bnh", hid, inputs["U_weight"].astype(np.float32))
        + (inputs["W_bias"] + inputs["U_bias"]).astype(np.float32)
    )
    in_maps = prep_in_maps(inputs["img_features"], inputs["hidden_state"], consts)
    nc = make_nc(B)
    last_err = None
    for attempt in range(3):
        try:
            res = run_bass_kernel_spmd(
                nc, in_maps, core_ids=list(range(B)), trace=trace, tmpdir=tmpdir
            )
            break
        except Exception as e:  # transient NRT_EXEC_UNIT_UNRECOVERABLE etc.
            last_err = e
            if "UNRECOVERABLE" not in str(e) and "UNAVAILABLE" not in str(e):
                raise
    else:
        raise last_err
    phi = np.stack([res.results[b]["phi"] for b in range(B)]).astype(np.float32)
    return phi, res


def kernel(**inputs) -> np.ndarray:
    phi, _ = run(inputs, trace=False)
    return phi


# revision 55
# speedup vs baseline: 1.1799x; 1.0047x over previous
"""Trainium2 Bass kernel for nn_Attention_53334903882008 (additive attention), v20.

Reference (per batch b):
  We  = img @ W^T + Wb;  Ue = (hid @ U^T + Ub) broadcast over T
  att = tanh(We + Ue);   e = att @ w + wb
  alpha = softmax_N(e);  phi = sum_n alpha * img      -> [B, T, D]

Sharding: data-parallel over B=8, one batch per NeuronCore; weights
replicated. Measured 106.4-107.5 us (baseline v9 bf16: 169.9 us),
rel err 0.019245 (gate 2e-2) — deterministic across runs.

How the 1.6x was won (v10..v20):
  - The We matmul (8192x1024x512 per core, 64% of PE work) runs FULLY
    in fp8-e4m3 with perf_mode=DoubleRow: each [128,2,*] MM contracts
    256 rows in one ~512-cycle stream, so We costs 4 streams instead
    of 8. W is pre-scaled by 16 host-side so its ~N(0,1/1024) entries
    clear the e4m3 subnormal range; the 1/16 descale is folded into
    the DVE op that adds U_comb.
  - The non-fp8 stack (xn, att, adiag, U_comb, w, base_log) is fp16
    rather than bf16 (3 more mantissa bits, same bytes): buys the
    quantization margin that makes full-fp8 We fit the 2e-2 gate
    (0.0192 vs 0.0194 simulated; both match HW to ~2e-5).
  - U_comb = hid @ U^T + Wb + Ub (0.3% of FLOPs) is computed on the
    host and shipped in the blob, killing 9 setup matmuls and the
    startup stall waiting for it.
  - Engine balance per tile: PE ~1.3 us (4 DR + 2 phi + s), DVE
    ~1.4 us queue-overlapped (descale+add-U stt writing SBUF f32 so
    the PSUM bank frees early, e-dot stt with accum), ACT ~1.1 us
    (tanh + exp). PSUM: 5 We banks + phi0/phi1/s.
  - HAM warmup: 12 garbage MMs keep the PE busy through the DMA
    spin-up so the clock gate reaches 8/8 by ~13 us and never drops.

DMA: ONE ring (sync), strict need-order FIFO. Cross-queue engine
arbitration is coarse: with two HWDGE rings, whichever has backlog
monopolizes the 16-engine pool in multi-us bursts and the other
stream's needed-now bytes stall the PE (v14/v15 traces). A single
FIFO ordered by first-need paces itself (~417 GB/s measured). Unit =
4 btn-tiles: xt 0.5 MB, xn 1 MB; buffer-granular deps wake the PE per
4 tiles. Startup pieces (2+2+4 tiles) are separate SBUF tiles so
We(0) starts after ~0.9 MB.

Per 128-row btn-tile (64 tiles):
  - PSUM[btn, h] = I128^T @ ucomb16 + sum_g xt8^T @ wt8 (4 fp8-e4m3
    DoubleRow MMs; W pre-scaled by 16 host-side to clear the e4m3
    subnormal range) = 16*(We + U_comb)
  - tanh on ACT with scale=1/16 -> att fp16
  - e column [128,1] via one fused DVE scalar_tensor_tensor
    (out = att * w_rep, accum_out = sum_h)
  - adiag = exp(base_log + e) in ONE ACT op (bias = e per-partition;
    base_log is 0 on the block-diag band, -30000 off-band) -> the
    block-diagonal unnormalized-softmax matrix directly
  - phi[t,d] += adiag^T @ xn (2 fp16 matmuls) and s[t] += adiag^T @
    ones (N=1 matmul) accumulate in persistent PSUM over all 64 tiles
  - final: phi *= 1/s, DMA out.
U_comb = hid @ U^T + Wb + Ub (0.3% of the FLOPs) is computed host-side
in f32 and shipped in the blob as 16*U_comb fp16.
"""

from contextlib import ExitStack

import numpy as np
import ml_dtypes

import concourse.bacc as bacc
import concourse.tile as tile
from concourse import mybir
from concourse.bass_utils import run_bass_kernel_spmd

B = 8

BF = mybir.dt.bfloat16
F16 = mybir.dt.float16
F8 = mybir.dt.float8e4
U8 = mybir.dt.uint8
F32 = mybir.dt.float32
NPBF = ml_dtypes.bfloat16
NPF8 = ml_dtypes.float8_e4m3
DR = mybir.MatmulPerfMode.DoubleRow

T, N, D, H = 128, 64, 1024, 512
BTN = T * N            # 8192
NI = BTN // 128        # 64 btn-tiles of 128 rows
KT = D // 128          # 8 contraction tiles over d
G8 = 4                 # DoubleRow double-k groups (all kt in fp8)
WS = 16.0              # host-side W scale (keeps W out of e4m3 subnormals)
CPT = 8                # btn-tiles per chunk (startup bookkeeping)
PHI_LAG = 3            # tiles between chain(i) and phi(i) in PE order
N_WARM = 7             # garbage warmup MMs (HAM warm + DMA spin-up cover)

TC = G8 * 2 * 128      # 1024 fp8 bytes per tile in xt

# blob column offsets (fp16 [128, BLOB_C])
O_UCOMB = 0                     # [128, 512]  hid@U^T + Wb + Ub
O_WREP = O_UCOMB + H            # [128, 512]  w replicated over partitions
O_BASE = O_WREP + H             # [128, 254]  base_log band
O_ONEC = O_BASE + 254           # [128, 1]    ones column
BLOB_C = O_ONEC + 1


def build(nc):
    xt_d = nc.dram_tensor("xt", [128, NI * TC], U8, kind="ExternalInput").ap()
    xn_d = nc.dram_tensor("xn", [128, NI * D], F16, kind="ExternalInput").ap()
    wt8_d = nc.dram_tensor("wt8", [128, G8 * 2 * H], F8, kind="ExternalInput").ap()
    blob_d = nc.dram_tensor("blob", [128, BLOB_C], F16, kind="ExternalInput").ap()
    phi_d = nc.dram_tensor("phi", [T, D], F32, kind="ExternalOutput").ap()

    with tile.TileContext(nc) as tc, ExitStack() as ctx:
        consts = ctx.enter_context(tc.tile_pool(name="consts", bufs=1))
        xtp = ctx.enter_context(tc.tile_pool(name="xt", bufs=6))
        xnp = ctx.enter_context(tc.tile_pool(name="xn", bufs=5))
        attp = ctx.enter_context(tc.tile_pool(name="att", bufs=3))
        smal = ctx.enter_context(tc.tile_pool(name="smalls", bufs=6))
        pswe = ctx.enter_context(tc.tile_pool(name="pswe", bufs=5, space="PSUM"))
        psph = ctx.enter_context(tc.tile_pool(name="psphi", bufs=1, space="PSUM"))

        # ---- ONE ring (sync), strict need-order FIFO ----
        NU = NI // 4       # 16 units of 4 btn-tiles

        # Startup order solved against need-times at ~400 GB/s:
        # [xt0p0, wt8, xt0p1, blob, xt0p2, xn0a, xtU2, xn0b, xtU3,
        #  xnU2, xtU4, xnU3, xtU5]
        xt0_pieces = []   # (tile, first_ig, ntiles)
        t0a = consts.tile([128, 2, TC], U8)
        nc.sync.dma_start(out=t0a, in_=xt_d[:, 0 : 2 * TC])
        xt0_pieces.append((t0a, 0, 2))
        wt8_r = wt8_d.rearrange("p (g i h) -> p g i h", g=G8, i=2)
        # two tiles so the first tile's DR MMs only wait on k-groups 0-1
        # (tile deps are buffer-granular)
        wt8a = consts.tile([128, 2, 2, H], F8)
        nc.sync.dma_start(out=wt8a, in_=wt8_r[:, 0:2])
        t0b = consts.tile([128, 2, TC], U8)
        nc.sync.dma_start(out=t0b, in_=xt_d[:, 2 * TC : 4 * TC])
        xt0_pieces.append((t0b, 2, 2))
        wt8b = consts.tile([128, 2, 2, H], F8)
        nc.sync.dma_start(out=wt8b, in_=wt8_r[:, 2:4])
        wt8g = [wt8a[:, 0], wt8a[:, 1], wt8b[:, 0], wt8b[:, 1]]
        blob = consts.tile([128, BLOB_C], F16)
        nc.sync.dma_start(out=blob, in_=blob_d)
        t0c = consts.tile([128, 4, TC], U8)
        nc.sync.dma_start(out=t0c, in_=xt_d[:, 4 * TC : 8 * TC])
        xt0_pieces.append((t0c, 4, 4))

        def emit_xt(u):  # unit u covers btn-tiles 4u..4u+3
            xtc = xtp.tile([128, 4, TC], U8, tag="xt")
            nc.sync.dma_start(out=xtc, in_=xt_d[:, 4 * u * TC : 4 * (u + 1) * TC])
            return xtc

        def emit_xn(u):
            xnc = xnp.tile([128, 4, D], F16, tag="xn")
            nc.sync.dma_start(out=xnc, in_=xn_d[:, 4 * u * D : 4 * (u + 1) * D])
            return xnc

        xn0_pieces = []
        n0 = consts.tile([128, 4, D], F16)
        nc.sync.dma_start(out=n0, in_=xn_d[:, 0 : 4 * D])
        xn0_pieces.append(n0)
        xt_bufs, xn_bufs = {}, {}
        xt_bufs[2] = emit_xt(2)
        n1 = consts.tile([128, 4, D], F16)
        nc.sync.dma_start(out=n1, in_=xn_d[:, 4 * D : 8 * D])
        xn0_pieces.append(n1)
        xt_bufs[3] = emit_xt(3)
        xn_bufs[2] = emit_xn(2)
        xt_bufs[4] = emit_xt(4)
        xn_bufs[3] = emit_xn(3)
        xt_bufs[5] = emit_xt(5)

        def xt_view(ig):
            if ig < CPT:
                for t0, first, n in xt0_pieces:
                    if first <= ig < first + n:
                        return t0, ig - first
            return xt_bufs[ig // 4], ig % 4

        def xn_view(ig):
            if ig < CPT:
                return xn0_pieces[ig // 4], ig % 4
            return xn_bufs[ig // 4], ig % 4

        scratch = consts.tile([128, H], BF)  # HAM warmup fuel
        nc.gpsimd.memset(scratch, 0.0)

        ucomb = blob[:, O_UCOMB : O_UCOMB + H]
        w_rep = blob[:, O_WREP : O_WREP + H]
        base_log = blob[:, O_BASE : O_BASE + 254]
        onescol = blob[:, O_ONEC : O_ONEC + 1]

        # ---- persistent accumulators ----
        ps_phi0 = psph.tile([T, 512], F32, tag="phi0")
        ps_phi1 = psph.tile([T, 512], F32, tag="phi1")
        ps_phi = [ps_phi0, ps_phi1]
        ps_s = psph.tile([T, 1], F32, tag="s")

        def emit_we(ig):
            xtc, j = xt_view(ig)
            ps = pswe.tile([128, H], F32, tag="we")
            x8 = xtc[:, j, :].bitcast(F8).rearrange("p (g i c) -> p g i c", g=G8, i=2)
            for g in range(G8):
                m = nc.tensor.matmul(
                    ps, lhsT=x8[:, g], rhs=wt8g[g],
                    start=(g == 0), stop=(g == G8 - 1), perf_mode=DR,
                )
            return ps, m

        def emit_chain(ig, ps):
            # ps holds 16*We; descale and add U_comb in one DVE op, writing
            # to SBUF f32 so the PSUM bank frees here (not after tanh).
            ws32 = attp.tile([128, H], F32, tag="ws32")
            nc.vector.scalar_tensor_tensor(
                out=ws32, in0=ps, scalar=1.0 / WS, in1=ucomb,
                op0=mybir.AluOpType.mult, op1=mybir.AluOpType.add,
            )
            att = attp.tile([128, H], F16, tag="att")
            nc.scalar.activation(att, ws32, mybir.ActivationFunctionType.Tanh)
            scr = attp.tile([128, H], F16, tag="scr")
            ecol = smal.tile([128, 1], F32, tag="ecol")
            nc.vector.scalar_tensor_tensor(
                out=scr, in0=att, scalar=1.0, in1=w_rep,
                op0=mybir.AluOpType.mult, op1=mybir.AluOpType.mult,
                accum_out=ecol,
            )
            adiag = smal.tile([128, 128], F16, tag="adiag")
            nc.scalar.activation(
                adiag,
                base_log[:, 126 - 2 * ig : 254 - 2 * ig],
                mybir.ActivationFunctionType.Exp,
                bias=ecol,
            )
            return adiag

        def emit_phi(item):
            ig, adiag = item
            xnc, j = xn_view(ig)
            # s first: the finalize's reciprocal only needs ps_s, so the
            # last tile's s-stop lands before its phi pair
            nc.tensor.matmul(
                ps_s, lhsT=adiag, rhs=onescol, start=(ig == 0), stop=(ig == NI - 1)
            )
            for dh in range(2):
                nc.tensor.matmul(
                    ps_phi[dh],
                    lhsT=adiag,
                    rhs=xnc[:, j, dh * 512 : (dh + 1) * 512],
                    start=(ig == 0), stop=(ig == NI - 1),
                )

        # ---- main pipeline ----
        ps_warm = pswe.tile([128, H], F32, tag="we")
        for _ in range(N_WARM):
            nc.tensor.matmul(ps_warm, lhsT=scratch[:, 0:128], rhs=scratch,
                             start=True, stop=True)
        phi_pend = []  # (ig, adiag) awaiting phi emission
        for ig in range(NI):
            if ig % 4 == 0:
                u = ig // 4
                if u + 6 < NU:
                    xt_bufs[u + 6] = emit_xt(u + 6)
                if 4 <= u + 4 < NU:
                    xn_bufs[u + 4] = emit_xn(u + 4)
            ps, m_we = emit_we(ig)
            phi_pend.append((ig, emit_chain(ig, ps)))
            maxlag = 6 if ig < 16 else (PHI_LAG if ig < NI - 4 else 2)
            while len(phi_pend) > maxlag:
                emit_phi(phi_pend.pop(0))
        for item in phi_pend:
            emit_phi(item)

        # ---- finalize: phi = ps_phi * (1/s_t) ----
        recip = smal.tile([128, 1], F32, tag="recip")
        nc.vector.reciprocal(recip, ps_s)
        phi_sb = consts.tile([T, D], F32)
        # the two 1/s scales run on different engines so they overlap
        nc.vector.tensor_scalar_mul(phi_sb[:, 0:512], ps_phi[0], recip)
        nc.scalar.activation(
            phi_sb[:, 512:1024], ps_phi[1],
            mybir.ActivationFunctionType.Copy, scale=recip,
        )
        for dh in range(2):
            nc.sync.dma_start(
                out=phi_d[:, dh * 512 : (dh + 1) * 512],
                in_=phi_sb[:, dh * 512 : (dh + 1) * 512],
            )

    return nc


def prep_consts(W_weight, W_bias, U_weight, U_bias, w_weight):
    def pack_T(M):  # [H, D] -> [128, KT, H] f32, [p, kt, h] = M[h, kt*128+p]
        return M.T.astype(np.float32).reshape(KT, 128, H).transpose(1, 0, 2)

    wkt = pack_T(W_weight) * WS            # [128, KT, H], scaled
    wt8 = np.ascontiguousarray(wkt).reshape(128, G8 * 2 * H)
    wt8 = np.clip(wt8, -240, 240).astype(NPF8)

    blob = np.zeros((128, BLOB_C), np.float32)
    # ucomb filled per-core in prep_in_maps
    blob[:, O_WREP : O_WREP + H] = w_weight[0][None, :]
    blob[:, O_BASE : O_BASE + 254] = -30000.0
    for p in range(128):
        blob[p, O_BASE + 126 + p // 64] = 0.0
    blob[:, O_ONEC] = 1.0
    return {"wt8": wt8, "_blob_f32": blob}


_NC_CACHE = {}


def make_nc(num_devices=B):
    if num_devices not in _NC_CACHE:
        nc = bacc.Bacc(
            "TRN2", target_bir_lowering=False, debug=False, num_devices=num_devices
        )
        build(nc)
        nc.compile()
        _NC_CACHE[num_devices] = nc
    return _NC_CACHE[num_devices]


def prep_in_maps(img_features, hidden_state, consts):
    maps = []
    for b in range(B):
        xb = np.asarray(img_features[b], dtype=np.float32).reshape(BTN, D)
        xn = np.ascontiguousarray(
            xb.astype(np.float16).reshape(NI, 128, D).transpose(1, 0, 2)
        ).reshape(128, NI * D)
        # xt[p, tile, kt, c] = x[tile*128+c, kt*128+p] in fp8 bytes
        xkt = xb.reshape(NI, 128, KT, 128)
        x8 = np.clip(xkt.transpose(3, 0, 2, 1), -240, 240).astype(NPF8)
        xt = np.ascontiguousarray(x8.reshape(128, NI * TC).view(np.uint8))
        blob = consts["_blob_f32"].copy()
        # U_comb[c, h] = hid[c%64] @ U^T + Wb + Ub, f32 on host
        uc = consts["_ucomb_all"][b]
        blob[:, O_UCOMB : O_UCOMB + H] = np.concatenate([uc, uc], axis=0)
        maps.append(
            {
                "xt": xt, "xn": xn, "wt8": consts["wt8"],
                "blob": blob.astype(np.float16),
            }
        )
    return maps


def run(inputs, trace=False, tmpdir=None):
    """Run the SPMD kernel; returns (phi [B,T,D] fp32, BassKernelResults)."""
    inputs = {k: np.asarray(v) for k, v in inputs.items()}
    consts = prep_consts(
        inputs["W_weight"], inputs["W_bias"], inputs["U_weight"], inputs["U_bias"],
        inputs["w_weight"],
    )
    # [B, 64, H] = hid[n, b] @ U^T + (Wb + Ub)
    hid = np.asarray(inputs["hidden_state"], dtype=np.float32)
    consts["_ucomb_all"] = (
        np.einsum("nbd,hd->bnh", hid, inputs["U_weight"].astype(np.float32))
        + (inputs["W_bias"] + inputs["U_bias"]).astype(np.float32)
    )
    in_maps = prep_in_maps(inputs["img_features"], inputs["hidden_state"], consts)
    nc = make_nc(B)
    last_err = None
    for attempt in range(3):
        try:
            res = run_bass_kernel_spmd(
                nc, in_maps, core_ids=list(range(B)), trace=trace, tmpdir=tmpdir
            )
            break
        except Exception as e:  # transient NRT_EXEC_UNIT_UNRECOVERABLE etc.
            last_err = e
            if "UNRECOVERABLE" not in str(e) and "UNAVAILABLE" not in str(e):
                raise
    else:
        raise last_err
    phi = np.stack([res.results[b]["phi"] for b in range(B)]).astype(np.float32)
    return phi, res


def kernel(**inputs) -> np.ndarray:
    phi, _ = run(inputs, trace=False)
    return phi
